# revision 1
# baseline (speedup 1.0000x reference)
"""Multi-head causal attention (B=2, S=2048, D=1024, H=16, dh=64) on 8 TRN2 cores.

Strategy
--------
- Shard the 32 (batch, head) pairs across 8 cores, 4 pairs each (cores 0-3: b=0,
  cores 4-7: b=1). Pure data parallel, no collectives.
- Per head, compute S^T = K @ Q^T directly on the PE (contraction over dh=64 on
  the partition axis), so softmax-exp output P^T = exp(S^T) is already in the
  [k, q] layout the P@V matmul needs as lhsT/rhs -- no on-device transposes.
- Softmax without max-subtraction (scores are O(1) after the 1/sqrt(dh) scale,
  exp never overflows in fp32; identical result up to fp rounding).
- Row sums l_q come for free from the P@V matmul by appending a ones-column to
  V ([2048, 65]); output row 64 of O^T accumulates sum_k P^T[k, q].
- Normalization (divide by l) and the final [65, S] -> [S, 64] transpose happen
  on the host, so the device writes O^T straight from PSUM.
- Two heads are packed per 128 SBUF partitions; their K=64-contraction S^T
  matmuls issue to disjoint PE row-groups (tile_position auto-derived from the
  base partition) and run concurrently on the systolic array.
- The mask is handled by host-side block planning at [128 k x 512 q]
  granularity: all-masked blocks are skipped, fully-kept blocks run unmasked,
  and mixed blocks get a 0/1 multiply from a small set of deduplicated mask
  tiles uploaded per core. For the causal mask this is exactly flash-style
  block skipping (~2x work saving) with a single unique diagonal tile.
- Every partial-width block (W < 512, i.e. the diagonal) fuses both heads'
  S^T matmuls into 128-contraction matmuls over a host-built zero-interleaved
  Q tensor (qz): head A's slice in rows 0-63 / cols [0:W], head B's in rows
  64-127 / cols [W:2W], zeros elsewhere. One matmul per spanned PSUM bank
  (single PE writer per bank -- HW-safe) lands both halves contiguously, so a
  single exp instruction covers them, cutting ACT instruction count ~25%.
- All matmuls use float32r: measured on HW at fp32-level accuracy (rel err
  1.6e-4 vs f64, identical to the fp32 path) at 4x the fp32 matmul rate.
"""

import os
import sys
from contextlib import ExitStack

import numpy as np

for _p in ("/opt/trn_rl_repo", "/root/.axon_site/_ro/trn_rl_repo"):
    if os.path.isdir(_p) and _p not in sys.path:
        sys.path.insert(0, _p)
        break

import concourse.bacc as bacc  # noqa: E402
import concourse.mybir as mybir  # noqa: E402
import concourse.tile as tile  # noqa: E402
from concourse.bass_utils import run_bass_kernel_spmd  # noqa: E402

F32 = mybir.dt.float32
F32R = mybir.dt.float32r
EXP = mybir.ActivationFunctionType.Exp

N_CORES = 8
H = 16
DH = 64
QBLK = 512
KBLK = 128

# persistent-SBUF budget for mask tiles; beyond this they stream from DRAM
MASK_SBUF_LIMIT = 64 * 1024  # bytes per partition

LAST_RESULTS = None  # BassKernelResults of the most recent kernel() call


def _plan_blocks(mask):
    """Classify [KBLK x QBLK] blocks of S^T per q-chunk, union over batch.

    Returns (plans, uniq_contents):
      plans[qc] = list of (kk, c0, c1, m0, m1, uid); block covers k rows
        kk*KBLK..+KBLK and q columns qc*QBLK+c0..qc*QBLK+c1. If uid >= 0,
        multiply P^T block columns [m0, m1) by mask tile `uid`.
      uniq_contents[uid] = float32 [B, KBLK, mw] 0/1 tile (per-batch content).
    The first block of each plan covers the whole column union so its matmul
    can own start=True for the PSUM accumulation group.
    """
    B, S, _ = mask.shape
    NQ, NK = S // QBLK, S // KBLK
    uniq_keys = {}
    uniq_contents = []
    plans = []
    for qc in range(NQ):
        raw = []
        for kk in range(NK):
            sub = mask[:, qc * QBLK:(qc + 1) * QBLK, kk * KBLK:(kk + 1) * KBLK]
            anyk = sub.any(axis=(0, 2))  # [QBLK] column needed?
            if not anyk.any():
                continue
            c0 = int(anyk.argmax()) & ~3
            c1 = min(QBLK, (QBLK - int(anyk[::-1].argmax()) + 3) & ~3)
            raw.append([kk, c0, c1])
        if not raw:
            plans.append([])
            continue
        C0 = min(b[1] for b in raw)
        C1 = max(b[2] for b in raw)
        fi = next((i for i, b in enumerate(raw) if b[1] == C0 and b[2] == C1),
                  None)
        if fi is None:
            raw[0][1], raw[0][2] = C0, C1  # extend block 0 to cover the union
            fi = 0
        raw.insert(0, raw.pop(fi))
        out = []
        for kk, c0, c1 in raw:
            sub = mask[:, qc * QBLK:(qc + 1) * QBLK, kk * KBLK:(kk + 1) * KBLK]
            allk = sub.all(axis=(0, 2))
            dirty = ~allk
            dirty[:c0] = False
            dirty[c1:] = False
            if dirty.any():
                m0 = int(dirty.argmax()) & ~3
                m1 = min(QBLK, (QBLK - int(dirty[::-1].argmax()) + 3) & ~3)
                dirty[m0:m1] = True  # widened cols join the masked region
                content = np.zeros((B, KBLK, m1 - m0), np.float32)
                for bb in range(B):
                    content[bb] = sub[bb, m0:m1, :].T
                key = content.tobytes()
                uid = uniq_keys.get(key)
                if uid is None:
                    uid = len(uniq_contents)
                    uniq_keys[key] = uid
                    uniq_contents.append(content)
            else:
                m0 = m1 = 0
                uid = -1
            out.append((kk, c0, c1, m0, m1, uid))
        plans.append(out)
    mw = max((c.shape[2] for c in uniq_contents), default=1)
    uniq_padded = []
    for c in uniq_contents:
        p = np.zeros((B, KBLK, mw), np.float32)
        p[:, :, :c.shape[2]] = c
        uniq_padded.append(p)
    return plans, uniq_padded


ZW_CAP = 6144  # max fused-staging columns (24 KB/partition x 2 bufs in SBUF)


def _plan_z(plans):
    """Assign qz column offsets to fusible blocks (2W <= QBLK), per q-chunk.

    Returns (zmap, zw, qcoffs): zmap[(qc, kk)] = column offset of that
    block's [128, 2W] zero-interleaved staging slice; qcoffs[qc] = (start,
    end) column range of chunk qc's slices (for chunked loading).
    """
    zmap = {}
    zw = 0
    qcoffs = []
    for qc, blocks in enumerate(plans):
        start = zw
        for kk, c0, c1, m0, m1, uid in blocks:
            W = c1 - c0
            # W < QBLK: the two halves aren't contiguous at QBLK offsets, so
            # fusing pays. 2W > QBLK just needs one matmul per spanned bank.
            if W < QBLK and zw + 2 * W <= ZW_CAP:
                zmap[(qc, kk)] = zw
                zw += 2 * W
        qcoffs.append((start, zw))
    return zmap, zw, qcoffs


def _build(S, n_groups, n_pairs, plans, n_uniq, zinfo, mw=1, repeat=1,
           la=2, p_bufs=6, s_bufs=3, o_bufs=1, osb_bufs=4):
    """Build the single SPMD program run identically on all cores.

    repeat > 1 re-runs the whole body (for wall-clock benchmarking only).
    """
    NQ, NK = S // QBLK, S // KBLK
    VW = DH + 1  # V with ones column
    nc = bacc.Bacc("TRN2", target_bir_lowering=False, debug=False)
    qt = nc.declare_dram_parameter("qt", [n_groups, 128, S], F32R, isOutput=False)
    kt = nc.declare_dram_parameter("kt", [n_groups, 128, S], F32R, isOutput=False)
    vv = nc.declare_dram_parameter("vv", [n_pairs, 128, NK * VW], F32R,
                                   isOutput=False)
    mk = nc.declare_dram_parameter("mk", [max(n_uniq, 1), 128, mw], F32R,
                                   isOutput=False)
    zmap, zw, qcoffs = zinfo
    qz = nc.declare_dram_parameter("qz", [n_groups, 128, max(zw, 1)], F32R,
                                   isOutput=False)
    ot = nc.declare_dram_parameter("ot", [n_pairs, VW, S], F32, isOutput=True)

    with tile.TileContext(nc) as tc, ExitStack() as ctx:
        qpool = ctx.enter_context(tc.tile_pool(name="qpool", bufs=2))
        kpool = ctx.enter_context(tc.tile_pool(name="kpool", bufs=2))
        vpool = ctx.enter_context(tc.tile_pool(name="vpool", bufs=3))
        mpool = ctx.enter_context(tc.tile_pool(name="mpool", bufs=1))
        ppool = ctx.enter_context(tc.tile_pool(name="ppool", bufs=p_bufs))
        obuf = ctx.enter_context(tc.tile_pool(name="obuf", bufs=osb_bufs))
        spool = ctx.enter_context(tc.tile_pool(name="spool", bufs=s_bufs, space="PSUM"))
        opool = ctx.enter_context(tc.tile_pool(name="opool", bufs=2, space="PSUM"))

        # Trigger the ACT exp-table load at t=0 so its ~2.7us overlaps the
        # initial input DMAs instead of delaying the first real exp.
        warm = mpool.tile([128, 8], F32)
        nc.vector.memset(warm[:], 0.0)
        nc.scalar.activation(warm[:], warm[:], EXP)

        # budget the persistent-mask decision against the qz staging
        # footprint (zw cols x 4 B x 2 bufs) -- both live in SBUF for the
        # whole kernel, and together they can overflow it (HW crash, unseen
        # by the allocator) even when each alone fits
        mask_budget = max(MASK_SBUF_LIMIT - 8 * zw, 16 * 1024)
        stream_masks = max(n_uniq, 1) * mw * 4 > mask_budget
        if not stream_masks:
            mtile = mpool.tile([128, max(n_uniq, 1) * mw], F32R)

        # Zero-interleaved rhs staging tiles for fused narrow blocks: head A's
        # Q slice sits in rows 0-63 / cols [0:W], head B's in rows 64-127 /
        # cols [W:2W], zeros elsewhere (memset once; DMAs never touch the
        # zero quadrants). One 128-contraction matmul then yields both heads'
        # S^T halves contiguously in a single PSUM bank -> one exp covers
        # both. One tile per distinct W keeps stale data out.


        first_group = True
        giter = [g for _ in range(repeat) for g in range(n_groups)]
        for gi, g in enumerate(giter):
            is_last_group = gi == len(giter) - 1
            ktile = kpool.tile([128, S], F32R, tag="kt")
            qtile = qpool.tile([128, S], F32R, tag="qt")
            vtiles = [vpool.tile([128, NK * VW], F32R, tag=f"vt{h}",
                                 name=f"vt{h}") for h in range(2)]
            # chunked loads, first-needed first: the opening S-matmuls only
            # need the leading columns, so don't serialize them behind
            # monolithic 1 MB transfers (DMA is bus-serial at ~330 GB/s)
            nq4 = max(NK // 4, 1) * VW  # V quarter: one qc's worth of kk
            nc.gpsimd.dma_start(ktile[:, 0:KBLK], kt[g, :, 0:KBLK])
            if zw:
                qztile = vpool.tile([128, zw], F32R, tag="qz", name="qztile",
                                    bufs=2)
                for z0, z1 in qcoffs:
                    if z0 < z1:
                        nc.gpsimd.dma_start(qztile[:, z0:z1], qz[g, :, z0:z1])
            nc.sync.dma_start(qtile[:, 0:QBLK], qt[g, :, 0:QBLK])
            nc.sync.dma_start(ktile[:, KBLK:QBLK], kt[g, :, KBLK:QBLK])
            if first_group:
                if not stream_masks:
                    for u in range(n_uniq):
                        nc.sync.dma_start(mtile[:, u * mw:(u + 1) * mw], mk[u])
                first_group = False
            for h in range(2):
                nc.sync.dma_start(vtiles[h][:, 0:nq4], vv[2 * g + h, :, 0:nq4])
            vdone = nq4
            for c0 in range(QBLK, S, QBLK):
                nc.sync.dma_start(ktile[:, c0:c0 + QBLK], kt[g, :, c0:c0 + QBLK])
                nc.sync.dma_start(qtile[:, c0:c0 + QBLK], qt[g, :, c0:c0 + QBLK])
                v1 = min(vdone + nq4, NK * VW)
                for h in range(2):
                    if vdone < v1:
                        nc.sync.dma_start(vtiles[h][:, vdone:v1],
                                          vv[2 * g + h, :, vdone:v1])
                vdone = v1
            for h in range(2):
                if vdone < NK * VW:
                    nc.sync.dma_start(vtiles[h][:, vdone:],
                                      vv[2 * g + h, :, vdone:])

            for qc in range(NQ):
                blocks = plans[qc]
                if not blocks:
                    continue
                if is_last_group and qc == NQ - 1 and len(blocks) > 2:
                    # the kernel drain runs: last exp -> (mask mul) -> last
                    # P@V -> copy -> store. Put masked/narrow blocks early in
                    # this final chunk so the drain chain is wide & DVE-free.
                    blocks = [blocks[0]] + sorted(
                        blocks[1:], key=lambda b: (b[5] < 0, b[2] - b[1]))
                nb = len(blocks)
                o_ps = [opool.tile([VW, QBLK], F32, tag=f"o{h}", name=f"o_ps{h}",
                                   bufs=o_bufs)
                        for h in range(2)]
                LA = la  # blocks of PE-lookahead before each P@V accumulate
                staged = []
                for i in range(nb + LA):
                    if i < nb:
                        kk, c0, c1, m0, m1, uid = blocks[i]
                        W = c1 - c0
                        # NOTE: TWO matmuls writing one PSUM bank (+ an ACT
                        # read) crashes real HW. The fused path below is safe:
                        # a single matmul writes the whole [0:2W] region.
                        zoff = zmap.get((qc, kk))
                        s_ps = spool.tile([128, 2 * QBLK], F32, tag="s")
                        p_t = ppool.tile([128, 2 * QBLK], F32R, tag="p")
                        q0 = qc * QBLK + c0
                        if zoff is not None:
                            hoff = W
                            # one matmul per spanned PSUM bank (single writer
                            # per bank -- the HW-safe pattern), one exp total
                            for ci in range(0, 2 * W, QBLK):
                                ce = min(ci + QBLK, 2 * W)
                                nc.tensor.matmul(
                                    s_ps[:, ci:ce],
                                    lhsT=ktile[:, kk * KBLK:(kk + 1) * KBLK],
                                    rhs=qztile[:, zoff + ci:zoff + ce],
                                    start=True, stop=True)
                            nc.scalar.activation(p_t[:, 0:2 * W],
                                                 s_ps[:, 0:2 * W], EXP)
                        else:
                            hoff = QBLK
                            for h in range(2):
                                nc.tensor.matmul(
                                    s_ps[:, h * QBLK:h * QBLK + W],
                                    lhsT=ktile[64 * h:64 * h + 64,
                                               kk * KBLK:(kk + 1) * KBLK],
                                    rhs=qtile[64 * h:64 * h + 64, q0:q0 + W],
                                    start=True, stop=True)
                            if W == QBLK:
                                nc.scalar.activation(p_t[:, 0:2 * QBLK],
                                                     s_ps[:, 0:2 * QBLK], EXP)
                            else:
                                for h in range(2):
                                    nc.scalar.activation(
                                        p_t[:, h * QBLK:h * QBLK + W],
                                        s_ps[:, h * QBLK:h * QBLK + W], EXP)
                        if uid >= 0:
                            if stream_masks:
                                ms = mpool.tile([128, mw], F32R, tag="ms",
                                                name="ms", bufs=4)
                                nc.sync.dma_start(ms[:, 0:m1 - m0],
                                                  mk[uid, :, 0:m1 - m0])
                                mop = ms[:, 0:m1 - m0]
                            else:
                                mop = mtile[:, uid * mw:uid * mw + (m1 - m0)]
                            for h in range(2):
                                lo = h * hoff + (m0 - c0)
                                nc.vector.tensor_mul(
                                    p_t[:, lo:lo + (m1 - m0)],
                                    p_t[:, lo:lo + (m1 - m0)], mop)
                        staged.append((i, kk, c0, c1, W, hoff, p_t))
                    if i >= LA:
                        j, kk, c0, c1, W, hoff, p_t = staged[i - LA]
                        for h in range(2):
                            nc.tensor.matmul(
                                o_ps[h][:, c0:c1],
                                lhsT=vtiles[h][:, kk * VW:(kk + 1) * VW],
                                rhs=p_t[:, h * hoff:h * hoff + W],
                                start=(j == 0), stop=(j == nb - 1))
                for h in range(2):
                    dst = ot[2 * g + h, :, qc * QBLK:(qc + 1) * QBLK]
                    osb = obuf.tile([VW, QBLK], F32, tag="osb")
                    if is_last_group and qc == NQ - 1:
                        # kernel drain path: copies in parallel on DVE + ACT
                        # (ACT is idle after the final exp), stores split over
                        # three DGEs so their latencies overlap
                        hq = QBLK // 2
                        if h == 0:
                            nc.vector.tensor_copy(osb[:], o_ps[h][:])
                            nc.sync.dma_start(dst, osb[:])
                        else:
                            nc.scalar.copy(osb[:], o_ps[h][:])
                            nc.gpsimd.dma_start(dst[:, 0:hq], osb[:, 0:hq])
                            nc.scalar.dma_start(dst[:, hq:], osb[:, hq:])
                    else:
                        nc.vector.tensor_copy(osb[:], o_ps[h][:])
                        nc.gpsimd.dma_start(dst, osb[:])
    nc.finalize()
    return nc


def _make_in_maps(q4, k4, v4, maskb, uniq, n_groups, per_core, zinfo,
                  plans):
    B, S = q4.shape[0], q4.shape[1]
    NK = S // KBLK
    VW = DH + 1
    n_uniq = len(uniq)
    zmap, zw, _ = zinfo
    in_maps = []
    for c in range(N_CORES):
        qt = np.empty((n_groups, 128, S), np.float32)
        kt = np.empty((n_groups, 128, S), np.float32)
        vvv = np.empty((per_core, 128, NK * VW), np.float32)
        bs = []
        for lp in range(per_core):
            gp = c * per_core + lp
            b, h = divmod(gp, H)
            bs.append(b)
            g, half = divmod(lp, 2)
            qt[g, 64 * half:64 * half + 64] = q4[b, :, h, :].T
            kt[g, 64 * half:64 * half + 64] = k4[b, :, h, :].T
            vt = np.ones((128, NK, VW), np.float32)
            vt[:, :, :DH] = v4[b, :, h, :].reshape(NK, KBLK, DH).transpose(1, 0, 2)
            vvv[lp] = vt.reshape(128, NK * VW)
        if n_uniq:
            assert len(set(bs)) == 1, "mask tiles assume one batch per core"
            mkarr = np.ascontiguousarray(
                np.stack([uniq[u][bs[0]] for u in range(n_uniq)]))
        else:
            mkarr = np.zeros((1, 128, 1), np.float32)
        qzarr = np.zeros((n_groups, 128, max(zw, 1)), np.float32)
        for qc, blocks in enumerate(plans):
            for kk, c0, c1, m0, m1, uid in blocks:
                zoff = zmap.get((qc, kk))
                if zoff is None:
                    continue
                W = c1 - c0
                q0 = qc * QBLK + c0
                qzarr[:, 0:64, zoff:zoff + W] = qt[:, 0:64, q0:q0 + W]
                qzarr[:, 64:128, zoff + W:zoff + 2 * W] = \
                    qt[:, 64:128, q0:q0 + W]
        in_maps.append({"qt": qt, "kt": kt, "vv": vvv, "mk": mkarr,
                        "qz": qzarr})
    return in_maps


def _assemble(results, B, S, per_core):
    D = H * DH
    out = np.empty((B, S, D), np.float32)
    for c in range(N_CORES):
        otc = results[c]["ot"]  # [per_core, DH+1, S]
        for lp in range(per_core):
            gp = c * per_core + lp
            b, h = divmod(gp, H)
            l = otc[lp, DH].astype(np.float64)
            l = np.where(l == 0.0, 1.0, l)
            out[b, :, h * DH:(h + 1) * DH] = \
                (otc[lp, :DH] / l).T.astype(np.float32)
    return out


def kernel(queries, keys, values, mask):
    B, S, D = queries.shape
    assert D == H * DH
    q4 = (np.ascontiguousarray(queries, dtype=np.float32) * 0.125) \
        .reshape(B, S, H, DH)
    k4 = np.ascontiguousarray(keys, dtype=np.float32).reshape(B, S, H, DH)
    v4 = np.ascontiguousarray(values, dtype=np.float32).reshape(B, S, H, DH)
    maskb = np.asarray(mask).astype(bool)

    plans, uniq = _plan_blocks(maskb)
    zinfo = _plan_z(plans)
    per_core = (B * H) // N_CORES
    n_groups = per_core // 2

    mw = uniq[0].shape[2] if uniq else 1
    nc = _build(S, n_groups, per_core, plans, len(uniq), zinfo, mw=mw)
    in_maps = _make_in_maps(q4, k4, v4, maskb, uniq, n_groups, per_core,
                            zinfo, plans)
    try:
        res = run_bass_kernel_spmd(nc, in_maps, core_ids=list(range(N_CORES)))
    except ModuleNotFoundError:
        # BASS_TRACE set but the axon NTFF profiling hook isn't installed in
        # this container -- rerun untraced
        os.environ["BASS_NEVER_TRACE"] = "1"
        res = run_bass_kernel_spmd(nc, in_maps, core_ids=list(range(N_CORES)))
    global LAST_RESULTS
    LAST_RESULTS = res
    return _assemble(res.results, B, S, per_core)



# revision 30
# speedup vs baseline: 1.3832x; 1.3832x over previous
"""Multi-head causal attention (B=2, S=2048, D=1024, H=16, dh=64) on 8 TRN2 cores.

Strategy
--------
- Shard the 32 (batch, head) pairs across 8 cores, 4 pairs each (cores 0-3: b=0,
  cores 4-7: b=1). Pure data parallel, no collectives.
- Per head, compute S^T = K @ Q^T directly on the PE (contraction over dh=64 on
  the partition axis), so softmax-exp output P^T = exp(S^T) is already in the
  [k, q] layout the P@V matmul needs -- no on-device transposes.
- All matmul operands are bf16 (rhs dtype sets the PE rate: 1 cyc/row flat, no
  fp32r narrow-AP penalty); PSUM accumulation stays fp32.
- Softmax without max-subtraction (scores are O(1) after the 1/sqrt(dh) scale,
  exp never overflows in fp32; identical result up to fp rounding).
- exp is THE bottleneck engine-wise (one ACT at 0.83 ns/col), so the score ->
  P conversion is SPLIT between ACT (true exp) and DVE (Schraudolph bit-trick:
  int16(x*128/ln2 + B) reinterpreted as bf16 ~= e^x to +-3%; softmax
  normalization via the shared ones-column cancels most of it -- measured end
  to end ~4e-3 max rel err with a ~50% split). A static greedy planner
  balances per-engine busy time (cols/rate + per-instruction overheads).
- Row sums l_q come for free from the P@V matmul by appending a ones-column to
  V ([2048, 65]); output row 64 of O^T accumulates sum_k P^T[k, q].
- Normalization (divide by l) and the final transpose happen on the host.
- Two heads are packed per 128 SBUF partitions; their K=64-contraction S^T
  matmuls issue to disjoint PE row-groups (tile_position auto-derived from the
  base partition) and run concurrently on the systolic array.
- The mask is handled by host-side block planning at [128 k x 512 q]
  granularity: all-masked blocks are skipped, fully-kept blocks run unmasked,
  and mixed blocks get a 0/1 multiply (both heads in one strided DVE
  instruction) from a small set of deduplicated bf16 mask tiles.
- PV_SWAP mode: P@V runs with P^T as the stationary operand and V moving, so
  the streamed dim is dh+1=65 instead of W -- output lands as O [q, 65] per
  128-q subblock, accumulated per (chunk, head) PSUM bank, and the host skips
  the transpose.
"""

import os
import sys
from contextlib import ExitStack

import numpy as np

for _p in ("/opt/trn_rl_repo", "/root/.axon_site/_ro/trn_rl_repo"):
    if os.path.isdir(_p) and _p not in sys.path:
        sys.path.insert(0, _p)
        break

import concourse.bacc as bacc  # noqa: E402
import concourse.mybir as mybir  # noqa: E402
import concourse.tile as tile  # noqa: E402
from concourse.bass_utils import run_bass_kernel_spmd  # noqa: E402

F32 = mybir.dt.float32
BF16 = mybir.dt.bfloat16
I16 = mybir.dt.int16
EXP = mybir.ActivationFunctionType.Exp
MULT = mybir.AluOpType.mult
ADD = mybir.AluOpType.add

N_CORES = 8
H = 16
DH = 64
QBLK = 512
KBLK = 128
VW = DH + 1

PV_SWAP = os.environ.get("K_PV_SWAP", "1") == "1"
USE_SCHRAUD = os.environ.get("K_SCHRAUD", "1") == "1"
POOL_MASK = os.environ.get("K_POOL_MASK", "1") == "1"

# Schraudolph constants for bf16: i16 = x * (2^7/ln2) + SCH_B, bitcast bf16.
SCH_A = 128.0 / float(np.log(2.0))
SCH_B = 16249.0

# engine-time model used only for the static exp/copy split (ns)
ACT_RATE, ACT_OVH = 0.833, 0.833 * 222 + 32 + 100 + float(os.environ.get("K_AOVH", "0"))
DVE_RATE, DVE_OVH = 1.042, 1.042 * 120 + 45 + 100

LAST_RESULTS = None  # BassKernelResults of the most recent kernel() call


def _plan_blocks(mask):
    """Classify [KBLK x QBLK] blocks of S^T per q-chunk, union over batch.

    Returns (plans, uniq_contents):
      plans[qc] = list of (kk, c0, c1, m0, m1, uid); block covers k rows
        kk*KBLK..+KBLK and q columns qc*QBLK+c0..qc*QBLK+c1. If uid >= 0,
        multiply P^T block columns [m0, m1) by mask tile `uid`.
      uniq_contents[uid] = float32 [B, KBLK, mw] 0/1 tile (per-batch content).
    """
    B, S, _ = mask.shape
    NQ, NK = S // QBLK, S // KBLK
    uniq_keys = {}
    uniq_contents = []
    plans = []
    for qc in range(NQ):
        out = []
        for kk in range(NK):
            sub = mask[:, qc * QBLK:(qc + 1) * QBLK, kk * KBLK:(kk + 1) * KBLK]
            anyk = sub.any(axis=(0, 2))  # [QBLK] column needed?
            if not anyk.any():
                continue
            c0 = int(anyk.argmax()) & ~3
            c1 = min(QBLK, (QBLK - int(anyk[::-1].argmax()) + 3) & ~3)
            if PV_SWAP:
                # swap-mode P@V slices lhsT at 128-aligned q-subblocks
                c0 &= ~(KBLK - 1)
                c1 = min(QBLK, (c1 + KBLK - 1) & ~(KBLK - 1))
            allk = sub.all(axis=(0, 2))
            dirty = ~allk
            dirty[:c0] = False
            dirty[c1:] = False
            if dirty.any():
                m0 = int(dirty.argmax()) & ~3
                m1 = min(QBLK, (QBLK - int(dirty[::-1].argmax()) + 3) & ~3)
                dirty[m0:m1] = True
                content = np.zeros((B, KBLK, m1 - m0), np.float32)
                for bb in range(B):
                    content[bb] = sub[bb, m0:m1, :].T
                key = content.tobytes()
                uid = uniq_keys.get(key)
                if uid is None:
                    uid = len(uniq_contents)
                    uniq_keys[key] = uid
                    uniq_contents.append(content)
            else:
                m0 = m1 = 0
                uid = -1
            out.append((kk, c0, c1, m0, m1, uid))
        plans.append(out)
    mw = max((c.shape[2] for c in uniq_contents), default=1)
    uniq_padded = []
    for c in uniq_contents:
        p = np.zeros((B, KBLK, mw), np.float32)
        p[:, :, :c.shape[2]] = c
        uniq_padded.append(p)
    return plans, uniq_padded


def _plan_engines(S, n_groups, plans):
    """Greedy-balance the per-block exp work (and out-copies) across ACT/DVE.

    Returns dict keyed (gi, qc, kk, h) -> 'a'|'v' for exp items (h=-1 means
    the fused full-width pair item) plus ('copy', gi, qc, h) -> 'a'|'v'.
    DVE is pre-loaded with the mask-multiply cost it always carries.
    """
    NQ = S // QBLK
    load = {"a": 0.0, "v": 0.0}
    assign = {}

    items = []
    for gi in range(n_groups):
        for qc in range(NQ):
            for (kk, c0, c1, m0, m1, uid) in plans[qc]:
                W = c1 - c0
                items.append(((gi, qc, kk, -1), 2 * W))
            ccols = 4 * VW if PV_SWAP else QBLK
            for h in range(2):
                items.append((("copy", gi, qc, h), ccols))

    for key, cols in items:
        ta = load["a"] + ACT_RATE * cols + ACT_OVH
        tv = load["v"] + DVE_RATE * cols + DVE_OVH
        if ta <= tv:
            assign[key] = "a"
            load["a"] = ta
        else:
            assign[key] = "v"
            load["v"] = tv
    return assign, load


def _build(S, n_groups, n_pairs, plans, n_uniq, mw=1, repeat=1,
           la=None, p_bufs=None, s_bufs=3, o_bufs=1, osb_bufs=None):
    if la is None:
        la = int(os.environ.get("K_LA", "2"))
    if p_bufs is None:
        p_bufs = int(os.environ.get("K_PBUFS", "6"))
    if osb_bufs is None:
        osb_bufs = int(os.environ.get("K_OSB", "4"))
    """Build the single SPMD program run identically on all cores."""
    NQ, NK = S // QBLK, S // KBLK
    nc = bacc.Bacc("TRN2", target_bir_lowering=False, debug=False)
    qt = nc.declare_dram_parameter("qt", [n_groups, 128, S], BF16, isOutput=False)
    kt = nc.declare_dram_parameter("kt", [n_groups, 128, S], BF16, isOutput=False)
    vv = nc.declare_dram_parameter("vv", [n_pairs, 128, NK * VW], BF16,
                                   isOutput=False)
    mk = nc.declare_dram_parameter("mk", [max(n_uniq, 1), 128, 2 * mw], BF16,
                                   isOutput=False)
    if PV_SWAP:
        ot = nc.declare_dram_parameter("ot", [n_groups, NQ, 128, 8 * VW], F32,
                                       isOutput=True)
    else:
        ot = nc.declare_dram_parameter("ot", [n_pairs, VW, S], F32, isOutput=True)

    engplan, _ = _plan_engines(S, n_groups, plans)

    with tile.TileContext(nc) as tc, ExitStack() as ctx:
        qpool = ctx.enter_context(tc.tile_pool(name="qpool", bufs=2))
        kpool = ctx.enter_context(tc.tile_pool(name="kpool", bufs=2))
        vpool = ctx.enter_context(tc.tile_pool(name="vpool", bufs=3))
        mpool = ctx.enter_context(tc.tile_pool(name="mpool", bufs=1))
        ppool = ctx.enter_context(tc.tile_pool(name="ppool", bufs=p_bufs))
        obuf = ctx.enter_context(tc.tile_pool(name="obuf", bufs=osb_bufs))
        spool = ctx.enter_context(tc.tile_pool(name="spool", bufs=s_bufs, space="PSUM"))
        opool = ctx.enter_context(tc.tile_pool(name="opool", bufs=2, space="PSUM"))

        warm = mpool.tile([128, 8], F32)
        warmb = mpool.tile([8, 8], BF16)
        mtile = mpool.tile([128, max(n_uniq, 1) * 2 * mw], BF16)

        def exp_emit(eng, dst, src):
            if eng == "a" or not USE_SCHRAUD:
                nc.scalar.activation(dst, src, EXP)
            else:
                nc.vector.tensor_scalar(dst.bitcast(I16), src, SCH_A, SCH_B,
                                        MULT, ADD)

        first_group = True
        giter = [g for _ in range(repeat) for g in range(n_groups)]
        for gi, g in enumerate(giter):
            is_last_group = gi == len(giter) - 1
            ktile = kpool.tile([128, S], BF16, tag="kt")
            qtile = qpool.tile([128, S], BF16, tag="qt")
            vtiles = [vpool.tile([128, NK * VW], BF16, tag=f"vt{h}",
                                 name=f"vt{h}") for h in range(2)]
            # first-needed-first: a small leading K/Q slice unblocks the
            # opening S-matmuls (split across two idle DGE queues at t=0),
            # then the bulk follows in one transfer each
            if first_group:
                nc.scalar.dma_start(qtile[:, 0:QBLK], qt[g, :, 0:QBLK])
                nc.sync.dma_start(ktile[:, 0:KBLK], kt[g, :, 0:KBLK])
                # ACT exp-table load overlaps the initial input DMAs
                nc.vector.memset(warm[:], 0.0)
                nc.scalar.activation(warm[:], warm[:], EXP)
                for u in range(n_uniq):
                    nc.gpsimd.dma_start(mtile[:, u * 2 * mw:(u + 1) * 2 * mw],
                                        mk[u])
                first_group = False
            else:
                nc.sync.dma_start(ktile[:, 0:KBLK], kt[g, :, 0:KBLK])
                nc.sync.dma_start(qtile[:, 0:QBLK], qt[g, :, 0:QBLK])
            nc.sync.dma_start(ktile[:, KBLK:S], kt[g, :, KBLK:S])
            for h in range(2):
                nc.sync.dma_start(vtiles[h][:], vv[2 * g + h])
            nc.sync.dma_start(qtile[:, QBLK:S], qt[g, :, QBLK:S])

            for qc in range(NQ):
                blocks = plans[qc]
                if not blocks:
                    continue
                drain = is_last_group and qc == NQ - 1
                if False and drain and PV_SWAP and len(blocks) > 2:
                    # keep masked/narrow blocks off the drain tail (the
                    # first emitted matmul's start lazily zeroes the whole
                    # output bank, so any order is safe in swap mode)
                    blocks = sorted(blocks,
                                    key=lambda b: (b[5] < 0, b[2] - b[1]))
                nb = len(blocks)
                if PV_SWAP:
                    o_ps = [opool.tile([128, 4 * VW], F32, tag=f"o{h}",
                                       name=f"o_ps{h}", bufs=o_bufs)
                            for h in range(2)]
                    # one accumulation group per (chunk, head) PSUM bank:
                    # start=True only on the first matmul into the bank (its
                    # lazy-zero covers the whole 2 KB region, so later
                    # subblock regions accumulate onto zero), stop=True on
                    # the bank's last matmul.
                    n_pv = sum((c1 - c0) // KBLK for kk, c0, c1, m0, m1, uid
                               in blocks)
                    pv_cnt = [0, 0]
                else:
                    o_ps = [opool.tile([VW, QBLK], F32, tag=f"o{h}",
                                       name=f"o_ps{h}", bufs=o_bufs)
                            for h in range(2)]
                LA = la  # blocks of PE-lookahead before each P@V accumulate
                staged = []
                for i in range(nb + LA):
                    if i < nb:
                        kk, c0, c1, m0, m1, uid = blocks[i]
                        W = c1 - c0
                        s_ps = spool.tile([128, 2 * QBLK], F32, tag="s")
                        p_t = ppool.tile([128, 2 * QBLK], BF16, tag="p")
                        q0 = qc * QBLK + c0
                        for h in range(2):
                            nc.tensor.matmul(
                                s_ps[:, h * QBLK + c0:h * QBLK + c1],
                                lhsT=ktile[64 * h:64 * h + 64,
                                           kk * KBLK:(kk + 1) * KBLK],
                                rhs=qtile[64 * h:64 * h + 64, q0:q0 + W],
                                start=True, stop=True)
                        eng = engplan[(g, qc, kk, -1)]
                        if W == QBLK:
                            if drain and i == nb - 1:
                                # drain tail: halve the final exp latency by
                                # splitting it across both engines
                                exp_emit("a", p_t[:, 0:QBLK], s_ps[:, 0:QBLK])
                                exp_emit("v", p_t[:, QBLK:2 * QBLK],
                                         s_ps[:, QBLK:2 * QBLK])
                            else:
                                exp_emit(eng, p_t[:, 0:2 * QBLK],
                                         s_ps[:, 0:2 * QBLK])
                        else:
                            # one strided instruction covers both heads'
                            # [c0, c1) slices (head stride QBLK)
                            sv = s_ps[:].rearrange("p (a q) -> p a q", a=2)
                            pv = p_t[:].rearrange("p (a q) -> p a q", a=2)
                            exp_emit(eng, pv[:, :, c0:c1], sv[:, :, c0:c1])
                        if uid >= 0:
                            # both heads in one strided Pool multiply; mask
                            # tile content is duplicated per head halves
                            pm = p_t[:].rearrange("p (a q) -> p a q", a=2)
                            pm = pm[:, :, m0:m1]
                            mm = mtile[:, uid * 2 * mw:uid * 2 * mw
                                       + 2 * (m1 - m0)]
                            mm = mm.rearrange("p (a b) -> p a b", a=2)
                            if POOL_MASK:
                                nc.gpsimd.tensor_mul(pm, pm, mm)
                            else:
                                nc.vector.tensor_mul(pm, pm, mm)
                        staged.append((i, kk, c0, c1, W, p_t))
                    if i >= LA:
                        j, kk, c0, c1, W, p_t = staged[i - LA]
                        if PV_SWAP:
                            for h in range(2):
                                for jj in range(c0 // KBLK, (c1 + KBLK - 1) // KBLK):
                                    je = min((jj + 1) * KBLK, c1)
                                    M = je - jj * KBLK
                                    pv_cnt[h] += 1
                                    nc.tensor.matmul(
                                        o_ps[h][0:M, jj * VW:(jj + 1) * VW],
                                        lhsT=p_t[:, h * QBLK + jj * KBLK:
                                                 h * QBLK + je],
                                        rhs=vtiles[h][:, kk * VW:(kk + 1) * VW],
                                        start=pv_cnt[h] == 1,
                                        stop=pv_cnt[h] == n_pv)
                        else:
                            for h in range(2):
                                nc.tensor.matmul(
                                    o_ps[h][:, c0:c1],
                                    lhsT=vtiles[h][:, kk * VW:(kk + 1) * VW],
                                    rhs=p_t[:, h * QBLK + c0:h * QBLK + c1],
                                    start=(j == 0), stop=(j == nb - 1))
                if PV_SWAP:
                    osb = obuf.tile([128, 8 * VW], F32, tag="osb")
                    for h in range(2):
                        eng = engplan[("copy", g, qc, h)]
                        dsl = osb[:, h * 4 * VW:(h + 1) * 4 * VW]
                        if eng == "a":
                            nc.scalar.copy(dsl, o_ps[h][:])
                        else:
                            nc.vector.tensor_copy(dsl, o_ps[h][:])
                    dst = ot[g, qc]
                    if is_last_group and qc == NQ - 1:
                        # drain: split the final store over two DGEs
                        nc.sync.dma_start(dst[:, 0:4 * VW], osb[:, 0:4 * VW])
                        nc.scalar.dma_start(dst[:, 4 * VW:], osb[:, 4 * VW:])
                    else:
                        # SWDGE queue: an out-store waiting on its copy must
                        # not block the SP queue head (input prefetches for
                        # the next group flow behind it)
                        nc.gpsimd.dma_start(dst, osb[:])
                else:
                    for h in range(2):
                        eng = engplan[("copy", g, qc, h)]
                        dst = ot[2 * g + h, :, qc * QBLK:(qc + 1) * QBLK]
                        osb = obuf.tile([VW, QBLK], F32, tag="osb")
                        if eng == "a":
                            nc.scalar.copy(osb[:], o_ps[h][:])
                        else:
                            nc.vector.tensor_copy(osb[:], o_ps[h][:])
                        if is_last_group and qc == NQ - 1:
                            hw = QBLK // 2
                            nc.sync.dma_start(dst[:, 0:hw], osb[:, 0:hw])
                            nc.scalar.dma_start(dst[:, hw:], osb[:, hw:])
                        else:
                            nc.gpsimd.dma_start(dst, osb[:])
    nc.finalize()
    return nc


def _make_in_maps(q4, k4, v4, maskb, uniq, n_groups, per_core):
    B, S = q4.shape[0], q4.shape[1]
    NK = S // KBLK
    n_uniq = len(uniq)
    mw = uniq[0].shape[2] if uniq else 1
    in_maps = []
    for c in range(N_CORES):
        qt = np.empty((n_groups, 128, S), np.float32)
        kt = np.empty((n_groups, 128, S), np.float32)
        vvv = np.empty((per_core, 128, NK * VW), np.float32)
        bs = []
        for lp in range(per_core):
            gp = c * per_core + lp
            b, h = divmod(gp, H)
            bs.append(b)
            g, half = divmod(lp, 2)
            qt[g, 64 * half:64 * half + 64] = q4[b, :, h, :].T
            kt[g, 64 * half:64 * half + 64] = k4[b, :, h, :].T
            vt = np.ones((128, NK, VW), np.float32)
            vt[:, :, :DH] = v4[b, :, h, :].reshape(NK, KBLK, DH).transpose(1, 0, 2)
            vvv[lp] = vt.reshape(128, NK * VW)
        if n_uniq:
            assert len(set(bs)) == 1, "mask tiles assume one batch per core"
            mkarr = np.zeros((n_uniq, 128, 2 * mw), np.float32)
            for u in range(n_uniq):
                mkarr[u, :, 0:mw] = uniq[u][bs[0]]
                mkarr[u, :, mw:2 * mw] = uniq[u][bs[0]]
        else:
            mkarr = np.zeros((1, 128, 2), np.float32)
        import ml_dtypes
        in_maps.append({
            "qt": qt.astype(ml_dtypes.bfloat16),
            "kt": kt.astype(ml_dtypes.bfloat16),
            "vv": vvv.astype(ml_dtypes.bfloat16),
            "mk": mkarr.astype(ml_dtypes.bfloat16),
        })
    return in_maps


def _assemble(results, B, S, per_core):
    D = H * DH
    out = np.empty((B, S, D), np.float32)
    for c in range(N_CORES):
        otc = results[c]["ot"]
        for lp in range(per_core):
            gp = c * per_core + lp
            b, h = divmod(gp, H)
            if PV_SWAP:
                g, half = divmod(lp, 2)
                # otc: [n_groups, NQ, 128, 2 (head), 4 (sub), VW]
                o = otc[g].reshape(S // QBLK, 128, 2, 4, VW)[:, :, half]
                o = o.transpose(0, 2, 1, 3).reshape(S, VW).astype(np.float64)
                l = o[:, DH]
                l = np.where(l == 0.0, 1.0, l)
                out[b, :, h * DH:(h + 1) * DH] = \
                    (o[:, :DH] / l[:, None]).astype(np.float32)
            else:
                l = otc[lp, DH].astype(np.float64)
                l = np.where(l == 0.0, 1.0, l)
                out[b, :, h * DH:(h + 1) * DH] = \
                    (otc[lp, :DH] / l).T.astype(np.float32)
    return out


def kernel(queries, keys, values, mask):
    B, S, D = queries.shape
    assert D == H * DH
    q4 = (np.ascontiguousarray(queries, dtype=np.float32) * 0.125) \
        .reshape(B, S, H, DH)
    k4 = np.ascontiguousarray(keys, dtype=np.float32).reshape(B, S, H, DH)
    v4 = np.ascontiguousarray(values, dtype=np.float32).reshape(B, S, H, DH)
    maskb = np.asarray(mask).astype(bool)

    plans, uniq = _plan_blocks(maskb)
    per_core = (B * H) // N_CORES
    n_groups = per_core // 2

    mw = uniq[0].shape[2] if uniq else 1
    nc = _build(S, n_groups, per_core, plans, len(uniq), mw=mw)
    in_maps = _make_in_maps(q4, k4, v4, maskb, uniq, n_groups, per_core)
    try:
        res = run_bass_kernel_spmd(nc, in_maps, core_ids=list(range(N_CORES)))
    except ModuleNotFoundError:
        os.environ["BASS_NEVER_TRACE"] = "1"
        res = run_bass_kernel_spmd(nc, in_maps, core_ids=list(range(N_CORES)))
    global LAST_RESULTS
    LAST_RESULTS = res
    return _assemble(res.results, B, S, per_core)
